# revision 1
# baseline (speedup 1.0000x reference)
"""DeepRC segment-softmax attention pooling kernel for 8 Trainium2 NeuronCores.

Strategy v2 (balanced shard, zero collectives, bf16 datapath):
  - N=131072 sorted instances split evenly: core c gets rows
    [c*16384, (c+1)*16384) -- no padding (vs per-bag pad to 18432 before).
    Bag boundaries fall inside at most one 512-subtile per boundary; those
    subtiles are split on host: kernel emits BOTH bagA-masked stats (zA,
    pooledA) and unmasked totals (z, pooled); host assigns B-side = tot - A.
  - xt is pre-transposed to [(l,c)=736 rows, 16384 cols] bf16; one core
    macrotile (2048 cols) loads with 2 batched DMAs (vs 6 unbatched f32).
  - Conv1d(K=32,C=23,KS=9,L=32->24) stays a banded matmul in bf16: 20
    (t,s) blocks of [128x128]; PSUM [128, 6, 256] per half-subtile.
  - maxpool over l: DVE reduces blocks 0-2, Pool max-chains blocks 3-5,
    DVE pair-maxes into bf16; partition fold 128->32 via Act copies + DVE
    bf16 maxes (2x DVE mode).
  - SELU split: Act relu + Act exp + min/add scalar_tensor_tensor on Pool;
    selu constant -lam*alpha deferred to host biases/output.
  - Attention logits use m=0 (|att| ~ O(1), exp safe in f32); host combine
    is exact in float64, so no per-subtile max is needed at all.
  - Attention MLP runs block-diagonal bf16 [128,512] per macrotile.
"""

import os
import sys

for _p in (
    "/root/.axon_site",
    "/root/.axon_site/_ro/trn_rl_repo",
    "/root/.axon_site/_ro/pypackages",
    "/opt/trn_rl_repo",
):
    if os.path.isdir(_p) and _p not in sys.path:
        sys.path.append(_p)

import numpy as np

import concourse.bass as bass
import concourse.mybir as mybir
from concourse.tile import TileContext, ScopedClock
from concourse.bass_utils import run_bass_kernel_spmd

AF = mybir.ActivationFunctionType
OP = mybir.AluOpType
AX = mybir.AxisListType
F32 = mybir.dt.float32
BF16 = mybir.dt.bfloat16
F8E4 = mybir.dt.float8e4
PM = mybir.MatmulPerfMode
NP_BF16 = mybir.dt.np(mybir.dt.bfloat16)
NP_F8 = mybir.dt.np(mybir.dt.float8e4)

# ---------------------------------------------------------------- constants
N_BAGS = 8
N_CORES = 8
L, C, K, U, KS = 32, 23, 32, 32, 9
LO = L - KS + 1            # 24 output positions
R = L * C                  # 736 rows of xT
NT = 6                     # PSUM M blocks (each 4 l x 32 k)
FD = 512                   # instances per subtile
HFD = 256                  # half-subtile (PSUM bank sizing)
QS = 4                     # subtiles stacked per macrotile
MACRO = QS * FD            # 2048

LAM = 1.0507009873554805
ALPHA = 1.6732632423543772
LA = LAM * ALPHA
LN_LA = float(np.log(LA))
C_SELU = -LA               # deferred selu constant

# ------------------------------------------------------- walrus workarounds


def _patched_drain_and_barrier(self, tick_clock, wait_clock):
    # stock version puts every outstanding sem wait on one drain; this
    # walrus build allows a single sync wait per instruction.
    nc = self.nc
    drain_inst = nc.sync.drain()
    wait_clock.add_sem_waits(
        drain_inst.ins, ScopedClock({None: tick_clock.global_clock})
    )
    si = drain_inst.ins.sync_info
    waits = list(si.on_wait or []) if si is not None else []
    if len(waits) > 1:
        si.on_wait = waits[:1]
        for w in waits[1:]:
            extra = nc.sync.drain()
            esi = extra.ins.sync_info
            if esi is None:
                extra.ins.sync_info = mybir.SyncInfo(on_wait=[w], on_update=[])
            else:
                esi.on_wait = [w]
    nc.all_engine_barrier()
    assert self.sems is not None
    popped = nc._tile_sem_poison_stack.pop()
    assert popped is self._sem_poison
    nc.clear_and_free_semaphores(list(self.sems.allocated().values()))
    nc.all_engine_barrier()


TileContext._drain_and_barrier = _patched_drain_and_barrier

_WSPLIT_CTR = [0]


def _split_multi_waits(nc):
    # move extra sem waits onto same-engine NoOps inserted just before the
    # owning instruction (equivalent gating, one wait per instruction).
    for func in nc.m.functions:
        for blk in func.blocks:
            out = []
            changed = False
            for inst in blk.instructions:
                si = inst.sync_info
                if si is not None and si.on_wait is not None and len(si.on_wait) > 1:
                    waits = list(si.on_wait)
                    for w in waits[:-1]:
                        _WSPLIT_CTR[0] += 1
                        nop = mybir.InstNoOp(
                            name=f"I-wsplit-{_WSPLIT_CTR[0]}", ins=[], outs=[]
                        )
                        nop.engine = inst.engine
                        nop.sync_info = mybir.SyncInfo(on_wait=[w], on_update=[])
                        out.append(nop)
                    si.on_wait = [waits[-1]]
                    changed = True
                out.append(inst)
            if changed:
                blk.instructions[:] = out
    return nc


# ------------------------------------------------------------- conv blocks


def _conv_block_list():
    """Nonzero (t, s) blocks of the banded weight matrix, t-major."""
    blocks = []
    for t in range(NT):
        lo_row = 23 * (4 * t)            # first needed row: l' = 4t
        hi_row = 23 * (4 * t + 12) + 22  # last needed row: l' = 4t+12, c=22
        s_lo, s_hi = lo_row // 128, hi_row // 128
        for s in range(s_lo, min(s_hi, 5) + 1):
            blocks.append((t, s))
    return blocks


CONV_BLOCKS = _conv_block_list()          # 20 blocks
N_CB = len(CONV_BLOCKS)

# fp8 DoubleRow conv at 16x weight scale with W-only residual correction:
#   16*w2t ~= W16 - Wr16, W16 = fp8(16 w2t), Wr16 = fp8(W16 - 16 w2t)
#   16*y   ~= W16 @ x8 - Wr16 @ x8   (x fp8 noise averages out downstream)
# The moving operand is a single x8 region (736 rows, 6 chunks of 128);
# each out-block t runs two 2-chunk-pair accumulation passes (W16, -Wr16)
# over rows [92t, 92t+276).
SCHUNKS = 6
MSTART = [(92 * t) // 256 for t in range(NT)]    # first chunk-pair per t
N_DR = 2                       # chunk-pairs per pass per out-block


def _build_w2t(conv_w):
    w2t = np.zeros((768, 768), np.float32)
    for l in range(LO):
        for j in range(KS):
            lp = l + j
            # rows 23*lp .. +23 ; cols 32*l .. +32 ; value w[k, c, j]
            w2t[23 * lp : 23 * lp + 23, 32 * l : 32 * l + 32] = conv_w[:, :, j].T
    return w2t


# --------------------------------------------------------------- program


def _build_program(NPAD):
    T = NPAD // MACRO
    nc = bass.Bass()
    sx_d = nc.declare_dram_parameter("sx", [128, SCHUNKS * NPAD], F8E4, isOutput=False)
    wconv_d = nc.declare_dram_parameter("wconv", [128, NT * 2 * N_DR * 256], F8E4, isOutput=False)
    wmat_d = nc.declare_dram_parameter("wmat", [128, 388], BF16, isOutput=False)
    wbias_d = nc.declare_dram_parameter("wbias", [128, 6], F32, isOutput=False)
    mask_d = nc.declare_dram_parameter("maskp", [QS, T * FD], BF16, isOutput=False)
    # single combined output: cols [0,T)=pooled, [T,2T)=pooledA,
    # cols [2T,3T) rows 0-3 = z, cols [3T,4T) rows 0-3 = zA
    out_d = nc.declare_dram_parameter("outs", [128, 4 * T], F32, isOutput=True)

    with TileContext(nc) as tc:
        with (
            tc.tile_pool(name="wpool", bufs=1) as wpool,
            tc.tile_pool(name="xpool", bufs=3) as xpool,
            tc.tile_pool(name="spool", bufs=3) as spool,
            tc.tile_pool(name="cpsum", bufs=2, space="PSUM") as cpsum,
            tc.tile_pool(name="mpsum", bufs=2, space="PSUM") as mpsum,
        ):
            # PE warm-up: the HAM clock gate needs ~3.4us of sustained PE
            # activity to release 2.4GHz; burn dummy matmuls on scratch data
            # while the first input DMAs are in flight.
            scratch = wpool.tile([128, 640], BF16)
            nc.gpsimd.memset(scratch[:], 0.0)
            for _ in range(6):
                wps = mpsum.tile([128, FD], F32, tag="mlp")
                nc.tensor.matmul(wps[:], scratch[:, 0:128], scratch[:, 128:640])

            wsb = wpool.tile([128, NT * 2 * N_DR * 256], F8E4)
            nc.sync.dma_start(wsb[:], wconv_d[:])
            wmat = wpool.tile([128, 388], BF16)
            nc.sync.dma_start(wmat[:], wmat_d[:])
            wbias = wpool.tile([128, 6], F32)
            nc.sync.dma_start(wbias[:], wbias_d[:])
            mask_sb = wpool.tile([QS, T * FD], BF16)
            nc.sync.dma_start(mask_sb[:], mask_d[:])
            outs_sb = wpool.tile([128, 4 * T], F32)
            nc.gpsimd.memset(outs_sb[:], 0.0)
            pooled_sb = outs_sb[:, 0:T]
            pooleda_sb = outs_sb[:, T : 2 * T]
            z_sb = outs_sb[0:QS, 2 * T : 3 * T]
            za_sb = outs_sb[0:QS, 3 * T : 4 * T]

            w1bd = wmat[:, 0:128]
            w2bd = wmat[:, 128:256]
            w3bd = wmat[:, 256:260]
            bc4 = wmat[0:4, 260:388]
            be_exp = wbias[:, 0:1]
            be_relu = wbias[:, 1:2]
            bh1_exp = wbias[:, 2:3]
            bh1_relu = wbias[:, 3:4]
            bh2_exp = wbias[:, 4:5]
            bh2_relu = wbias[:, 5:6]

            for j in range(T):
                xts = xpool.tile([128, SCHUNKS, MACRO], F8E4, tag="xts")
                col0 = j * MACRO
                # partition-outer strided DMAs, one per subtile so the first
                # conv can start ~1us after the load begins (512B elem runs)
                for qq in range(QS):
                    nc.sync.dma_start(
                        xts[:, :, qq * FD : (qq + 1) * FD],
                        sx_d[:].rearrange("p (s f) -> p s f", s=SCHUNKS)[
                            :, :, col0 + qq * FD : col0 + (qq + 1) * FD
                        ],
                    )
                er4 = spool.tile([128, FD], BF16, tag="er4")
                f1s = []
                for q in range(QS):
                    bm = spool.tile([128, FD], BF16, tag="bm")
                    for cc in range(2):
                        c0 = q * FD + cc * HFD
                        ps = cpsum.tile([128, NT, HFD], F32, tag="cps")
                        for t in range(NT):
                            for pi in range(2 * N_DR):   # (pass, mi) flat
                                m = MSTART[t] + (pi % N_DR)
                                idx = t * 2 * N_DR + pi
                                nc.tensor.matmul(
                                    ps[:, t, :],
                                    wsb[
                                        :, idx * 256 : (idx + 1) * 256
                                    ].rearrange("p (i m) -> p i m", i=2),
                                    xts[:, 2 * m : 2 * m + 2, c0 : c0 + HFD],
                                    start=(pi == 0),
                                    stop=(pi == 2 * N_DR - 1),
                                    perf_mode=PM.DoubleRow,
                                )
                        # max over the 6 l-blocks: GPSIMD cannot read PSUM,
                        # so Act exports blocks to SBUF bf16 and DVE merges
                        # at 2x; halves alternate the DVE/Act split to
                        # balance both engines.
                        bmo = bm[:, cc * HFD : (cc + 1) * HFD]
                        if cc == 0:
                            # DVE reduces blocks 0-1, Act copies 2-5
                            c4 = spool.tile([128, 4, HFD], BF16, tag="c4")
                            nc.scalar.copy(c4[:], ps[:, 2:6, :])
                            bmr = spool.tile([128, HFD], BF16, tag="bmr")
                            nc.vector.tensor_reduce(
                                bmr[:],
                                ps[:, 0:2, :].rearrange("p t f -> p f t"),
                                axis=AX.X, op=OP.max,
                            )
                            d2 = spool.tile([128, 2, HFD], BF16, tag="d2")
                            nc.vector.tensor_tensor(
                                d2[:], c4[:, 0:2, :], c4[:, 2:4, :], op=OP.max
                            )
                            d1 = spool.tile([128, HFD], BF16, tag="d1")
                            nc.vector.tensor_max(d1[:], d2[:, 0, :], d2[:, 1, :])
                            nc.vector.tensor_max(bmo, bmr[:], d1[:])
                        else:
                            # Act copies all 6 blocks, DVE merges the tree
                            c6 = spool.tile([128, 6, HFD], BF16, tag="c6")
                            nc.scalar.copy(c6[:], ps[:])
                            e3 = spool.tile([128, 3, HFD], BF16, tag="e3")
                            nc.vector.tensor_tensor(
                                e3[:], c6[:, 0:3, :], c6[:, 3:6, :], op=OP.max
                            )
                            e1 = spool.tile([128, HFD], BF16, tag="e1")
                            nc.vector.tensor_max(e1[:], e3[:, 0, :], e3[:, 1, :])
                            nc.vector.tensor_max(bmo, e1[:], e3[:, 2, :])
                    # partition fold stage 1: 128 -> 64
                    t64 = spool.tile([64, FD], BF16, tag="t64")
                    nc.gpsimd.tensor_copy(t64[:], bm[64:128, :])
                    f1 = spool.tile([64, FD], BF16, tag=f"f1{q}")
                    nc.vector.tensor_max(f1[:], bm[0:64, :], t64[:])
                    f1s.append(f1)
                # fold stage 2 deferred: the Pool copies of subtile q overlap
                # DVE merge work of subtile q+1 instead of serializing
                for q, f1 in enumerate(f1s):
                    t32 = spool.tile([32, FD], BF16, tag=f"t32{q % 2}")
                    nc.gpsimd.tensor_copy(t32[:], f1[32:64, :])
                    nc.vector.tensor_max(
                        er4[32 * q : 32 * q + 32, :], f1[0:32, :], t32[:]
                    )

                # ---- selu(er4/16 + conv_b): the fp8 conv runs at 16x scale,
                # folded into the activation scale (maxpool commutes)
                t_relu = spool.tile([128, FD], F32, tag="t_relu")
                nc.scalar.activation(
                    t_relu[:], er4[:], AF.Relu, bias=be_relu, scale=LAM / 16.0
                )
                v_exp = spool.tile([128, FD], F32, tag="v_exp")
                nc.scalar.activation(
                    v_exp[:], er4[:], AF.Exp, bias=be_exp, scale=1.0 / 16.0
                )
                e4 = spool.tile([128, FD], BF16, tag="e4")
                nc.vector.scalar_tensor_tensor(
                    e4[:], v_exp[:], LA, t_relu[:], op0=OP.min, op1=OP.add
                )
                # ---- MLP layer 1
                ps1 = mpsum.tile([128, FD], F32, tag="mlp")
                nc.tensor.matmul(ps1[:], w1bd, e4[:])
                t1 = spool.tile([128, FD], F32, tag="t1")
                nc.scalar.activation(t1[:], ps1[:], AF.Relu, bias=bh1_relu, scale=LAM)
                v1 = spool.tile([128, FD], F32, tag="v1")
                nc.scalar.activation(v1[:], ps1[:], AF.Exp, bias=bh1_exp, scale=1.0)
                h1 = spool.tile([128, FD], BF16, tag="h1")
                nc.vector.scalar_tensor_tensor(
                    h1[:], v1[:], LA, t1[:], op0=OP.min, op1=OP.add
                )
                # ---- MLP layer 2
                ps2 = mpsum.tile([128, FD], F32, tag="mlp")
                nc.tensor.matmul(ps2[:], w2bd, h1[:])
                t2 = spool.tile([128, FD], F32, tag="t2")
                nc.scalar.activation(t2[:], ps2[:], AF.Relu, bias=bh2_relu, scale=LAM)
                v2 = spool.tile([128, FD], F32, tag="v2")
                nc.scalar.activation(v2[:], ps2[:], AF.Exp, bias=bh2_exp, scale=1.0)
                h2 = spool.tile([128, FD], BF16, tag="h2")
                nc.vector.scalar_tensor_tensor(
                    h2[:], v2[:], LA, t2[:], op0=OP.min, op1=OP.add
                )
                # ---- attention logits (b3 cancels in softmax; m == 0)
                psa = mpsum.tile([4, FD], F32, tag="mlp")
                nc.tensor.matmul(psa[:], w3bd, h2[:])
                pexp = spool.tile([4, FD], BF16, tag="pexp")
                nc.scalar.activation(
                    pexp[:], psa[:], AF.Exp, bias=0.0, scale=1.0,
                    accum_out=z_sb[:, j : j + 1],
                )
                p4a = spool.tile([4, FD], BF16, tag="p4a")
                nc.vector.scalar_tensor_tensor(
                    p4a[:], pexp[:], 1.0, mask_sb[:, j * FD : (j + 1) * FD],
                    op0=OP.mult, op1=OP.mult, accum_out=za_sb[:, j : j + 1],
                )
                # ---- pooled(+A) += e4 * broadcast(p) per subtile group
                psbT = mpsum.tile([128, FD], F32, tag="mlp")
                nc.tensor.matmul(psbT[:], bc4, pexp[:])
                psbA = mpsum.tile([128, FD], F32, tag="mlp")
                nc.tensor.matmul(psbA[:], bc4, p4a[:])
                weT = spool.tile([128, FD], F32, tag="weT")
                nc.vector.scalar_tensor_tensor(
                    weT[:], e4[:], 1.0, psbT[:],
                    op0=OP.mult, op1=OP.mult, accum_out=pooled_sb[:, j : j + 1],
                )
                weA = spool.tile([128, FD], F32, tag="weA")
                nc.vector.scalar_tensor_tensor(
                    weA[:], e4[:], 1.0, psbA[:],
                    op0=OP.mult, op1=OP.mult, accum_out=pooleda_sb[:, j : j + 1],
                )

            nc.sync.dma_start(out_d[:], outs_sb[:])

    _split_multi_waits(nc)
    return nc


_PROGRAM_CACHE = {}
LAST_RESULTS = None  # set by kernel(); test.py reads trace/exec info


def _get_program(NPAD):
    if NPAD not in _PROGRAM_CACHE:
        _PROGRAM_CACHE[NPAD] = _build_program(NPAD)
    return _PROGRAM_CACHE[NPAD]


# ----------------------------------------------------------------- kernel


def kernel(
    inputs,
    segment_ids,
    conv_w,
    conv_b,
    att_w1,
    att_b1,
    att_w2,
    att_b2,
    att_w3,
    att_b3,
    out_w,
    out_b,
):
    global LAST_RESULTS
    x = np.asarray(inputs, np.float32)
    seg = np.asarray(segment_ids)
    conv_w = np.asarray(conv_w, np.float32)
    conv_b = np.asarray(conv_b, np.float32)
    att_w1 = np.asarray(att_w1, np.float32)
    att_b1 = np.asarray(att_b1, np.float32)
    att_w2 = np.asarray(att_w2, np.float32)
    att_b2 = np.asarray(att_b2, np.float32)
    att_w3 = np.asarray(att_w3, np.float32)
    att_b3 = np.asarray(att_b3, np.float32)
    out_w = np.asarray(out_w, np.float32)
    out_b = np.asarray(out_b, np.float32)

    n_total = x.shape[0]
    NPAD = -(-n_total // (N_CORES * MACRO)) * MACRO   # per-core cols
    T = NPAD // MACRO
    n_padded = N_CORES * NPAD
    n_pad = n_padded - n_total

    # ---------------- weights (shared by all cores)
    # fp8 DoubleRow conv at 16x scale, W-only residual (see constants above)
    w2t = _build_w2t(conv_w)
    w16 = 16.0 * w2t[:R]                               # [736, 768]
    W16 = w16.astype(NP_F8)
    Wr16neg = (w16 - W16.astype(np.float32)).astype(NP_F8)  # = -(W16 - 16w2t)
    passes = []
    for Wp in (W16, Wr16neg):
        sw = np.zeros((SCHUNKS * 128, 768), NP_F8)
        sw[:R] = Wp
        passes.append(sw)
    wconv8 = np.zeros((128, NT * 2 * N_DR * 256), NP_F8)
    for t in range(NT):
        for pi in range(2 * N_DR):
            m = MSTART[t] + (pi % N_DR)
            sw = passes[pi // N_DR]
            idx = t * 2 * N_DR + pi
            blk = sw[256 * m : 256 * (m + 1), 128 * t : 128 * (t + 1)]
            wconv8[:, idx * 256 : (idx + 1) * 256] = np.ascontiguousarray(
                blk.reshape(2, 128, 128).transpose(1, 0, 2).reshape(128, 256)
            )

    b1p = att_b1 + C_SELU * (att_w1 @ np.ones(K, np.float32))
    b2p = att_b2 + C_SELU * (att_w2 @ np.ones(U, np.float32))

    wmat = np.zeros((128, 388), np.float32)
    wbias = np.zeros((128, 6), np.float32)
    for q in range(QS):
        sl = slice(32 * q, 32 * q + 32)
        wmat[sl, 0:128][:, sl] = att_w1.T          # w1bd
        wmat[sl, 128:256][:, sl] = att_w2.T        # w2bd
        wmat[sl, 256 + q] = att_w3[0]              # w3bd
        wmat[q, 260 + 32 * q : 260 + 32 * q + 32] = 1.0  # bc4
        wbias[sl, 0] = conv_b + LN_LA
        wbias[sl, 1] = LAM * conv_b
        wbias[sl, 2] = b1p + LN_LA
        wbias[sl, 3] = LAM * b1p
        wbias[sl, 4] = b2p + LN_LA
        wbias[sl, 5] = LAM * b2p
    wmat16 = wmat.astype(NP_BF16)

    # ---------------- per-core inputs + bag bookkeeping
    xf = x.reshape(n_total, R)
    seg_pad = np.concatenate([seg, np.full(n_pad, N_BAGS, seg.dtype)])
    in_maps = []
    bagA = np.zeros((N_CORES, T, QS), np.int64)
    bagB = np.full((N_CORES, T, QS), -1, np.int64)
    npad_sub = np.zeros((N_CORES, T, QS), np.int64)
    for c in range(N_CORES):
        s0 = c * NPAD
        xt = np.zeros((R, NPAD), np.float32)
        real = min(NPAD, max(0, n_total - s0))
        if real > 0:
            xt[:, :real] = xf[s0 : s0 + real].T
        sxr = np.zeros((SCHUNKS * 128, NPAD), NP_F8)
        sxr[:R] = xt.astype(NP_F8)
        sx = np.ascontiguousarray(
            sxr.reshape(SCHUNKS, 128, NPAD).transpose(1, 0, 2).reshape(
                128, SCHUNKS * NPAD
            )
        )
        ids = seg_pad[s0 : s0 + NPAD].reshape(T, QS, FD)
        first = ids[:, :, 0]
        # real-instance last bag per subtile (pad slots hold N_BAGS)
        real_mask = ids < N_BAGS
        last_real = np.where(real_mask, ids, -1).max(axis=2)
        if ((last_real - first) > 1).any() and (last_real >= 0).all():
            raise ValueError("subtile spans >2 bags; unsupported input shape")
        bagA[c] = np.where(first < N_BAGS, first, -1)
        hasB = (last_real > first) & (first < N_BAGS)
        bagB[c] = np.where(hasB, last_real, -1)
        npad_sub[c] = (~real_mask).sum(axis=2)
        maskA = (ids == first[:, :, None]).astype(np.float32)
        maskA *= real_mask  # pad slots excluded even if first is pad
        # layout [QS, T*FD]
        maskp = np.ascontiguousarray(
            maskA.transpose(1, 0, 2).reshape(QS, T * FD)
        ).astype(NP_BF16)
        in_maps.append(
            {
                "sx": sx,
                "wconv": wconv8,
                "wmat": wmat16,
                "wbias": wbias,
                "maskp": maskp,
            }
        )

    nc = _get_program(NPAD)
    trace_mode = int(os.environ.get("DEEPRC_TRACE", "0"))
    kwargs = {}
    if trace_mode == 1:
        kwargs = dict(trace=True, trace_cores=[0])
    elif trace_mode >= 2:
        kwargs = dict(trace=True, trace_cores=list(range(N_CORES)), stitch_traces=True)
    res = run_bass_kernel_spmd(
        nc,
        in_maps,
        core_ids=list(range(N_CORES)),
        **kwargs,
    )
    LAST_RESULTS = res

    # ---------------- pad-instance constants (host, float64-exact path)
    # a zero input row gives er4 = 0 (conv of zeros, max of zeros), so the
    # device computes for each pad slot: e_pad = selu(conv_b)+LA etc.  The
    # unmasked totals z/pooled include those; subtract exactly here.
    if n_pad > 0:
        e_pad = np.where(
            conv_b > 0, LAM * conv_b, LA * np.exp(conv_b) - LA
        ) + LA  # selu(conv_b) + LA, shape [K]
        hh = np.where(
            att_w1 @ (e_pad + C_SELU) + att_b1 > 0,
            LAM * (att_w1 @ (e_pad + C_SELU) + att_b1),
            LA * np.exp(att_w1 @ (e_pad + C_SELU) + att_b1) - LA,
        )
        hh2 = np.where(
            att_w2 @ hh + att_b2 > 0,
            LAM * (att_w2 @ hh + att_b2),
            LA * np.exp(att_w2 @ hh + att_b2) - LA,
        )
        att_pad = float(att_w3[0] @ hh2)
        pz_pad = float(np.exp(att_pad))
        ppool_pad = pz_pad * e_pad  # [K]
    else:
        pz_pad = 0.0
        ppool_pad = np.zeros(K)

    # ---------------- exact host combine (float64)
    Z = np.zeros(N_BAGS, np.float64)
    P = np.zeros((N_BAGS, K), np.float64)
    for c in range(N_CORES):
        r = res.results[c]
        outs = r["outs"].astype(np.float64)          # [128, 4T]
        pooled = outs[:, 0:T].reshape(QS, K, T)
        pooleda = outs[:, T : 2 * T].reshape(QS, K, T)
        z = outs[0:QS, 2 * T : 3 * T]                # [4, T]
        za = outs[0:QS, 3 * T : 4 * T]
        for j in range(T):
            for q in range(QS):
                bA = bagA[c, j, q]
                if bA < 0:
                    continue
                bB = bagB[c, j, q]
                if bB < 0:
                    Z[bA] += za[q, j]
                    P[bA] += pooleda[q, :, j]
                else:
                    Z[bA] += za[q, j]
                    P[bA] += pooleda[q, :, j]
                    npd = npad_sub[c, j, q]
                    Z[bB] += z[q, j] - za[q, j] - npd * pz_pad
                    P[bB] += (
                        pooled[q, :, j] - pooleda[q, :, j] - npd * ppool_pad
                    )

    out = np.zeros((N_BAGS, 1), np.float32)
    for b in range(N_BAGS):
        pooled_bag = P[b] / Z[b] + C_SELU
        out[b, 0] = np.float32(
            float(out_w.astype(np.float64)[0] @ pooled_bag) + float(out_b[0])
        )
    return out



# revision 19
# speedup vs baseline: 1.1047x; 1.1047x over previous
"""DeepRC segment-softmax attention pooling kernel for 8 Trainium2 NeuronCores.

Strategy v3 (single-pass fp8 conv, fold-carrying PSUM pair-max reduce):
  - N=131072 sorted instances split evenly: core c gets rows
    [c*16384, (c+1)*16384). Bag boundaries fall inside at most one
    512-subtile per boundary; kernel emits BOTH bagA-masked stats (zA,
    pooledA) and unmasked totals (z, pooled); host assigns B-side = tot - A.
  - Conv1d(K=32,C=23,KS=9,L=32->24) is a banded matmul in fp8 DoubleRow at
    16x weight scale, SINGLE pass (no residual): rel err ~8.5e-3 << 2e-2.
  - The 24-way maxpool (6 PSUM l-blocks x 4 partition l-groups) runs as:
      L1: one DVE tensor_tensor max ps[:,0:3,:] vs ps[:,3:6,:] ->
          [128,3,256] bf16 (reads all 1536 psum elems, charged only 768).
      L2: two [128,256] maxes -> bm half; partition folds 128->64->32.
    Some halves route L1 through an Act c6 export + bf16 tree instead to
    balance Act/DVE/Pool occupancy (per-half engine assignment knobs).
  - SELU split: Act relu + Act exp + min/add scalar_tensor_tensor; selu
    constant -lam*alpha deferred to host biases/output.
  - Attention logits use m=0 (|att| ~ O(1)); host combine exact in float64.
"""

import os
import sys

for _p in (
    "/root/.axon_site",
    "/root/.axon_site/_ro/trn_rl_repo",
    "/root/.axon_site/_ro/pypackages",
    "/opt/trn_rl_repo",
):
    if os.path.isdir(_p) and _p not in sys.path:
        sys.path.append(_p)

import numpy as np

import concourse.bass as bass
import concourse.mybir as mybir
from concourse.tile import TileContext, ScopedClock
from concourse.bass_utils import run_bass_kernel_spmd

AF = mybir.ActivationFunctionType
OP = mybir.AluOpType
AX = mybir.AxisListType
F32 = mybir.dt.float32
BF16 = mybir.dt.bfloat16
F8E4 = mybir.dt.float8e4
PM = mybir.MatmulPerfMode
NP_BF16 = mybir.dt.np(mybir.dt.bfloat16)
NP_F8 = mybir.dt.np(mybir.dt.float8e4)

# ---------------------------------------------------------------- constants
N_BAGS = 8
N_CORES = 8
L, C, K, U, KS = 32, 23, 32, 32, 9
LO = L - KS + 1            # 24 output positions
R = L * C                  # 736 rows of xT
NT = 6                     # PSUM M blocks (each 4 l x 32 k)
FD = 512                   # instances per subtile
HFD = int(os.environ.get("DEEPRC_HFD", "256"))  # conv PSUM chunk width
NCHK = FD // HFD           # conv chunks per subtile
CPBUFS = int(os.environ.get("DEEPRC_CPBUFS", "2"))
SPBUFS = int(os.environ.get("DEEPRC_SPBUFS", "4"))
QS = 4                     # subtiles stacked per macrotile
MACRO = QS * FD            # 2048

LAM = 1.0507009873554805
ALPHA = 1.6732632423543772
LA = LAM * ALPHA
LN_LA = float(np.log(LA))
C_SELU = -LA               # deferred selu constant

# --------------------------------------------------- engine assignment knobs
# Engine legality (walrus verifier): tensor ALU ops run on DVE only (Pool
# rejects TensorTensor: NCC_IXCG966); Pool does copies/memset/ISA; both-PSUM
# tensor_tensor is illegal (NCC_IBVF027: one PSUM input max); two SBUF
# inputs must share the base partition (NCC_IBIR297).
# per half (q*2+cc, 8 total): 'A' = Act c6 export + DVE bf16 tree,
# 'B' = DVE direct 6-block tensor_reduce, 'H' = Act c3 export (blocks 3:6)
# + DVE pair-max vs psum blocks 0:3, 'J' = like H but the c3 export goes
# through a PSUM->SBUF DMA (f32) instead of Act.
CFG_PATHS = os.environ.get("DEEPRC_PATHS", "AHAAHAAH")
# L2 tail engine per half: 'v' = DVE ('g' illegal for ALU; kept for tuning)
CFG_L2 = os.environ.get("DEEPRC_L2", "vvvvvvvv")
# e3 (bf16 pair-merge after Act export) engine per half (A halves only)
CFG_E3 = os.environ.get("DEEPRC_E3", "vvvvvvvv")
# subtile fold engine (4)
CFG_FOLD = os.environ.get("DEEPRC_FOLD", "vvvv")
# selu combines for e4, h1, h2
CFG_COMB = os.environ.get("DEEPRC_COMB", "vvv")
# p4a engine
CFG_P4A = os.environ.get("DEEPRC_P4A", "v")

# ------------------------------------------------------- walrus workarounds


def _patched_drain_and_barrier(self, tick_clock, wait_clock):
    # stock version puts every outstanding sem wait on one drain; this
    # walrus build allows a single sync wait per instruction.
    nc = self.nc
    drain_inst = nc.sync.drain()
    wait_clock.add_sem_waits(
        drain_inst.ins, ScopedClock({None: tick_clock.global_clock})
    )
    si = drain_inst.ins.sync_info
    waits = list(si.on_wait or []) if si is not None else []
    if len(waits) > 1:
        si.on_wait = waits[:1]
        for w in waits[1:]:
            extra = nc.sync.drain()
            esi = extra.ins.sync_info
            if esi is None:
                extra.ins.sync_info = mybir.SyncInfo(on_wait=[w], on_update=[])
            else:
                esi.on_wait = [w]
    nc.all_engine_barrier()
    assert self.sems is not None
    popped = nc._tile_sem_poison_stack.pop()
    assert popped is self._sem_poison
    nc.clear_and_free_semaphores(list(self.sems.allocated().values()))
    nc.all_engine_barrier()


TileContext._drain_and_barrier = _patched_drain_and_barrier

_WSPLIT_CTR = [0]


def _split_multi_waits(nc):
    # move extra sem waits onto same-engine NoOps inserted just before the
    # owning instruction (equivalent gating, one wait per instruction).
    for func in nc.m.functions:
        for blk in func.blocks:
            out = []
            changed = False
            for inst in blk.instructions:
                si = inst.sync_info
                if si is not None and si.on_wait is not None and len(si.on_wait) > 1:
                    waits = list(si.on_wait)
                    for w in waits[:-1]:
                        _WSPLIT_CTR[0] += 1
                        nop = mybir.InstNoOp(
                            name=f"I-wsplit-{_WSPLIT_CTR[0]}", ins=[], outs=[]
                        )
                        nop.engine = inst.engine
                        nop.sync_info = mybir.SyncInfo(on_wait=[w], on_update=[])
                        out.append(nop)
                    si.on_wait = [waits[-1]]
                    changed = True
                out.append(inst)
            if changed:
                blk.instructions[:] = out
    return nc


# ------------------------------------------------------------- conv blocks

# fp8 DoubleRow conv at 16x weight scale, single pass:
#   16*w2t ~= fp8(16 w2t); the moving operand is a single x8 region
#   (736 rows, 6 chunks of 128); each out-block t accumulates 2 chunk-pair
#   matmuls over rows [256*MSTART[t], +512) \supseteq [92t, 92t+276).
SCHUNKS = 6
MSTART = [(92 * t) // 256 for t in range(NT)]    # first chunk-pair per t
N_PI = 2                       # chunk-pairs per out-block


def _build_w2t(conv_w):
    w2t = np.zeros((768, 768), np.float32)
    for l in range(LO):
        for j in range(KS):
            lp = l + j
            # rows 23*lp .. +23 ; cols 32*l .. +32 ; value w[k, c, j]
            w2t[23 * lp : 23 * lp + 23, 32 * l : 32 * l + 32] = conv_w[:, :, j].T
    return w2t


# --------------------------------------------------------------- program


def _eng(nc, ch):
    return nc.vector if ch == "v" else nc.gpsimd


def _build_program(NPAD):
    T = NPAD // MACRO
    nc = bass.Bass()
    sx_d = nc.declare_dram_parameter("sx", [128, SCHUNKS * NPAD], F8E4, isOutput=False)
    wconv_d = nc.declare_dram_parameter("wconv", [128, NT * N_PI * 256], F8E4, isOutput=False)
    wmat_d = nc.declare_dram_parameter("wmat", [128, 388], BF16, isOutput=False)
    wbias_d = nc.declare_dram_parameter("wbias", [128, 6], F32, isOutput=False)
    mask_d = nc.declare_dram_parameter("maskp", [QS, T * FD], BF16, isOutput=False)
    # single combined output: cols [0,T)=pooled, [T,2T)=pooledA,
    # cols [2T,3T) rows 0-3 = z, cols [3T,4T) rows 0-3 = zA
    out_d = nc.declare_dram_parameter("outs", [128, 4 * T], F32, isOutput=True)

    with TileContext(nc) as tc:
        with (
            tc.tile_pool(name="wpool", bufs=1) as wpool,
            tc.tile_pool(name="xpool", bufs=3) as xpool,
            tc.tile_pool(name="spool", bufs=SPBUFS) as spool,
            tc.tile_pool(name="cpsum", bufs=CPBUFS, space="PSUM") as cpsum,
            tc.tile_pool(name="mpsum", bufs=1, space="PSUM") as mpsum,
        ):
            # PE warm-up: the HAM clock gate needs ~3.4us of sustained PE
            # activity to release 2.4GHz; burn dummy matmuls on scratch data
            # while the first input DMAs are in flight.
            scratch = wpool.tile([128, 640], BF16)
            nc.gpsimd.memset(scratch[:], 0.0)
            for _ in range(6):
                wps = mpsum.tile([128, FD], F32, tag="mlp0")
                nc.tensor.matmul(wps[:], scratch[:, 0:128], scratch[:, 128:640])

            wsb = wpool.tile([128, NT * N_PI * 256], F8E4)
            nc.sync.dma_start(wsb[:], wconv_d[:])
            wmat = wpool.tile([128, 388], BF16)
            nc.sync.dma_start(wmat[:], wmat_d[:])
            wbias = wpool.tile([128, 6], F32)
            nc.sync.dma_start(wbias[:], wbias_d[:])
            mask_sb = wpool.tile([QS, T * FD], BF16)
            nc.sync.dma_start(mask_sb[:], mask_d[:])
            outs_sb = wpool.tile([128, 4 * T], F32)
            nc.gpsimd.memset(outs_sb[:], 0.0)
            pooled_sb = outs_sb[:, 0:T]
            pooleda_sb = outs_sb[:, T : 2 * T]
            z_sb = outs_sb[0:QS, 2 * T : 3 * T]
            za_sb = outs_sb[0:QS, 3 * T : 4 * T]

            w1bd = wmat[:, 0:128]
            w2bd = wmat[:, 128:256]
            w3bd = wmat[:, 256:260]
            bc4 = wmat[0:4, 260:388]
            be_exp = wbias[:, 0:1]
            be_relu = wbias[:, 1:2]
            bh1_exp = wbias[:, 2:3]
            bh1_relu = wbias[:, 3:4]
            bh2_exp = wbias[:, 4:5]
            bh2_relu = wbias[:, 5:6]

            # -------- software-pipelined macrotile loop: the MLP/attention
            # chain of macrotile j-1 is emitted interleaved with the conv
            # subtiles of macrotile j, and even/odd macrotiles use separate
            # single-buffer PSUM tags so adjacent chains overlap.
            def emit_mlp_stage(st, stage):
                jj = st["j"]
                tg = f"mlp{jj % 2}"
                if stage == 0:
                    ps1 = mpsum.tile([128, FD], F32, tag=tg, name=f"ps1_{jj}")
                    nc.tensor.matmul(ps1[:], w1bd, st["e4"][:])
                    t1 = spool.tile([128, FD], F32, tag="t1", name=f"t1_{jj}")
                    nc.scalar.activation(
                        t1[:], ps1[:], AF.Relu, bias=bh1_relu, scale=LAM
                    )
                    v1 = spool.tile([128, FD], F32, tag="v1", name=f"v1_{jj}")
                    nc.scalar.activation(
                        v1[:], ps1[:], AF.Exp, bias=bh1_exp, scale=1.0
                    )
                    h1 = spool.tile([128, FD], BF16, tag="h1", name=f"h1_{jj}")
                    _eng(nc, CFG_COMB[1]).scalar_tensor_tensor(
                        h1[:], v1[:], LA, t1[:], op0=OP.min, op1=OP.add
                    )
                    st["h1"] = h1
                elif stage == 1:
                    ps2 = mpsum.tile([128, FD], F32, tag=tg, name=f"ps2_{jj}")
                    nc.tensor.matmul(ps2[:], w2bd, st["h1"][:])
                    t2 = spool.tile([128, FD], F32, tag="t2", name=f"t2_{jj}")
                    nc.scalar.activation(
                        t2[:], ps2[:], AF.Relu, bias=bh2_relu, scale=LAM
                    )
                    v2 = spool.tile([128, FD], F32, tag="v2", name=f"v2_{jj}")
                    nc.scalar.activation(
                        v2[:], ps2[:], AF.Exp, bias=bh2_exp, scale=1.0
                    )
                    h2 = spool.tile([128, FD], BF16, tag="h2", name=f"h2_{jj}")
                    _eng(nc, CFG_COMB[2]).scalar_tensor_tensor(
                        h2[:], v2[:], LA, t2[:], op0=OP.min, op1=OP.add
                    )
                    st["h2"] = h2
                elif stage == 2:
                    psa = mpsum.tile([4, FD], F32, tag=tg, name=f"psa_{jj}")
                    nc.tensor.matmul(psa[:], w3bd, st["h2"][:])
                    pexp = spool.tile([4, FD], BF16, tag="pexp", name=f"pexp_{jj}")
                    nc.scalar.activation(
                        pexp[:], psa[:], AF.Exp, bias=0.0, scale=1.0,
                        accum_out=z_sb[:, jj : jj + 1],
                    )
                    p4a = spool.tile([4, FD], BF16, tag="p4a", name=f"p4a_{jj}")
                    _eng(nc, CFG_P4A).scalar_tensor_tensor(
                        p4a[:], pexp[:], 1.0,
                        mask_sb[:, jj * FD : (jj + 1) * FD],
                        op0=OP.mult, op1=OP.mult,
                        accum_out=za_sb[:, jj : jj + 1],
                    )
                    st["pexp"] = pexp
                    st["p4a"] = p4a
                else:
                    psbT = mpsum.tile([128, FD], F32, tag=tg, name=f"psbT_{jj}")
                    nc.tensor.matmul(psbT[:], bc4, st["pexp"][:])
                    weT = spool.tile([128, FD], F32, tag="weT", name=f"weT_{jj}")
                    nc.vector.scalar_tensor_tensor(
                        weT[:], st["e4"][:], 1.0, psbT[:],
                        op0=OP.mult, op1=OP.mult,
                        accum_out=pooled_sb[:, jj : jj + 1],
                    )
                    psbA = mpsum.tile([128, FD], F32, tag=tg, name=f"psbA_{jj}")
                    nc.tensor.matmul(psbA[:], bc4, st["p4a"][:])
                    weA = spool.tile([128, FD], F32, tag="weA", name=f"weA_{jj}")
                    nc.vector.scalar_tensor_tensor(
                        weA[:], st["e4"][:], 1.0, psbA[:],
                        op0=OP.mult, op1=OP.mult,
                        accum_out=pooleda_sb[:, jj : jj + 1],
                    )

            prev = None
            for j in range(T):
                xts = xpool.tile([128, SCHUNKS, MACRO], F8E4, tag="xts")
                col0 = j * MACRO
                # partition-outer strided DMAs, one per subtile so the first
                # conv can start ~1us after the load begins (512B elem runs)
                for qq in range(QS):
                    nc.sync.dma_start(
                        xts[:, :, qq * FD : (qq + 1) * FD],
                        sx_d[:].rearrange("p (s f) -> p s f", s=SCHUNKS)[
                            :, :, col0 + qq * FD : col0 + (qq + 1) * FD
                        ],
                    )
                er4 = spool.tile([128, FD], BF16, tag="er4")
                for q in range(QS):
                    bm = spool.tile([128, FD], BF16, tag="bm")
                    for cc in range(NCHK):
                        half = (q * NCHK + cc) % 8
                        path = CFG_PATHS[half]
                        c0 = q * FD + cc * HFD
                        ps = cpsum.tile([128, NT, HFD], F32, tag="cps")
                        # for H, blocks 3..5 first so the Act export starts
                        # while PE still works on blocks 0..2
                        t_order = (3, 4, 5, 0, 1, 2) if path == "H" else range(NT)
                        for t in t_order:
                            for pi in range(N_PI):
                                m = MSTART[t] + pi
                                idx = t * N_PI + pi
                                nc.tensor.matmul(
                                    ps[:, t, :],
                                    wsb[
                                        :, idx * 256 : (idx + 1) * 256
                                    ].rearrange("p (i m) -> p i m", i=2),
                                    xts[:, 2 * m : 2 * m + 2, c0 : c0 + HFD],
                                    start=(pi == 0),
                                    stop=(pi == N_PI - 1),
                                    perf_mode=PM.DoubleRow,
                                )
                        # 24-way max over 6 psum l-blocks (then partition
                        # folds 128->32 per subtile below)
                        l2e = _eng(nc, CFG_L2[half])
                        bmo = bm[:, cc * HFD : (cc + 1) * HFD]
                        if path == "B":
                            # one DVE reduce reads all 6 blocks
                            nc.vector.tensor_reduce(
                                bmo, ps[:].rearrange("p t f -> p f t"),
                                axis=AX.X, op=OP.max,
                            )
                        elif path in ("H", "J"):
                            # blocks 3:6 exported by Act (H) or by a
                            # PSUM->SBUF DMA in f32 (J, frees Act); DVE
                            # pair-maxes them against blocks 0:3 (single
                            # PSUM input: legal)
                            if path == "H":
                                c3 = spool.tile([128, 3, HFD], BF16, tag="c3")
                                nc.scalar.copy(c3[:], ps[:, 3:6, :])
                            else:
                                c3 = spool.tile([128, 3, HFD], F32, tag="c3j")
                                nc.sync.dma_start(c3[:], ps[:, 3:6, :])
                            t3 = spool.tile([128, 3, HFD], BF16, tag="t3")
                            nc.vector.tensor_tensor(
                                t3[:], ps[:, 0:3, :], c3[:], op=OP.max
                            )
                            m2 = spool.tile([128, HFD], BF16, tag="m2")
                            l2e.tensor_tensor(
                                m2[:], t3[:, 0, :], t3[:, 1, :], op=OP.max
                            )
                            l2e.tensor_tensor(bmo, m2[:], t3[:, 2, :], op=OP.max)
                        else:
                            c6 = spool.tile([128, 6, HFD], BF16, tag="c6")
                            nc.scalar.copy(c6[:], ps[:])
                            e3 = spool.tile([128, 3, HFD], BF16, tag="e3")
                            _eng(nc, CFG_E3[half]).tensor_tensor(
                                e3[:], c6[:, 0:3, :], c6[:, 3:6, :], op=OP.max
                            )
                            m2 = spool.tile([128, HFD], BF16, tag="m2")
                            l2e.tensor_tensor(
                                m2[:], e3[:, 0, :], e3[:, 1, :], op=OP.max
                            )
                            l2e.tensor_tensor(bmo, m2[:], e3[:, 2, :], op=OP.max)
                    # partition fold 128 -> 64 -> 32 into er4 rows [32q, +32).
                    # Two-input ops need equal input base partitions
                    # (NCC_IBIR297), so the upper halves are re-based via
                    # SBUF->SBUF DMA (DMA engines are otherwise idle).
                    t64 = spool.tile([64, FD], BF16, tag="t64")
                    nc.sync.dma_start(t64[:], bm[64:128, :])
                    f1 = spool.tile([64, FD], BF16, tag="f1")
                    _eng(nc, CFG_FOLD[q]).tensor_tensor(
                        f1[:], bm[0:64, :], t64[:], op=OP.max
                    )
                    t32 = spool.tile([32, FD], BF16, tag="t32")
                    nc.sync.dma_start(t32[:], f1[32:64, :])
                    _eng(nc, CFG_FOLD[q]).tensor_tensor(
                        er4[32 * q : 32 * q + 32, :],
                        f1[0:32, :], t32[:], op=OP.max,
                    )
                    # interleave one MLP/attention stage of macrotile j-1
                    if prev is not None:
                        emit_mlp_stage(prev, q)

                # ---- selu(er4/16 + conv_b): the fp8 conv runs at 16x scale,
                # folded into the activation scale (maxpool commutes)
                t_relu = spool.tile([128, FD], F32, tag="t_relu")
                nc.scalar.activation(
                    t_relu[:], er4[:], AF.Relu, bias=be_relu, scale=LAM / 16.0
                )
                v_exp = spool.tile([128, FD], F32, tag="v_exp")
                nc.scalar.activation(
                    v_exp[:], er4[:], AF.Exp, bias=be_exp, scale=1.0 / 16.0
                )
                e4 = spool.tile([128, FD], BF16, tag="e4")
                _eng(nc, CFG_COMB[0]).scalar_tensor_tensor(
                    e4[:], v_exp[:], LA, t_relu[:], op0=OP.min, op1=OP.add
                )
                prev = {"j": j, "e4": e4}

            # drain the last macrotile's MLP/attention chain
            for stage in range(4):
                emit_mlp_stage(prev, stage)

            nc.sync.dma_start(out_d[:], outs_sb[:])

    _split_multi_waits(nc)
    return nc


_PROGRAM_CACHE = {}
LAST_RESULTS = None  # set by kernel(); test.py reads trace/exec info


def _get_program(NPAD):
    if NPAD not in _PROGRAM_CACHE:
        _PROGRAM_CACHE[NPAD] = _build_program(NPAD)
    return _PROGRAM_CACHE[NPAD]


# ----------------------------------------------------------------- kernel


def kernel(
    inputs,
    segment_ids,
    conv_w,
    conv_b,
    att_w1,
    att_b1,
    att_w2,
    att_b2,
    att_w3,
    att_b3,
    out_w,
    out_b,
):
    global LAST_RESULTS
    x = np.asarray(inputs, np.float32)
    seg = np.asarray(segment_ids)
    conv_w = np.asarray(conv_w, np.float32)
    conv_b = np.asarray(conv_b, np.float32)
    att_w1 = np.asarray(att_w1, np.float32)
    att_b1 = np.asarray(att_b1, np.float32)
    att_w2 = np.asarray(att_w2, np.float32)
    att_b2 = np.asarray(att_b2, np.float32)
    att_w3 = np.asarray(att_w3, np.float32)
    att_b3 = np.asarray(att_b3, np.float32)
    out_w = np.asarray(out_w, np.float32)
    out_b = np.asarray(out_b, np.float32)

    n_total = x.shape[0]
    NPAD = -(-n_total // (N_CORES * MACRO)) * MACRO   # per-core cols
    T = NPAD // MACRO
    n_padded = N_CORES * NPAD
    n_pad = n_padded - n_total

    # ---------------- weights (shared by all cores)
    # fp8 conv at 16x scale, single pass (see constants above)
    w2t = _build_w2t(conv_w)
    w16 = 16.0 * w2t[:R]                               # [736, 768]
    sw = np.zeros((SCHUNKS * 128, 768), NP_F8)
    sw[:R] = w16.astype(NP_F8)
    wconv8 = np.zeros((128, NT * N_PI * 256), NP_F8)
    for t in range(NT):
        for pi in range(N_PI):
            m = MSTART[t] + pi
            idx = t * N_PI + pi
            blk = sw[256 * m : 256 * (m + 1), 128 * t : 128 * (t + 1)]
            wconv8[:, idx * 256 : (idx + 1) * 256] = np.ascontiguousarray(
                blk.reshape(2, 128, 128).transpose(1, 0, 2).reshape(128, 256)
            )

    b1p = att_b1 + C_SELU * (att_w1 @ np.ones(K, np.float32))
    b2p = att_b2 + C_SELU * (att_w2 @ np.ones(U, np.float32))

    wmat = np.zeros((128, 388), np.float32)
    wbias = np.zeros((128, 6), np.float32)
    for q in range(QS):
        sl = slice(32 * q, 32 * q + 32)
        wmat[sl, 0:128][:, sl] = att_w1.T          # w1bd
        wmat[sl, 128:256][:, sl] = att_w2.T        # w2bd
        wmat[sl, 256 + q] = att_w3[0]              # w3bd
        wmat[q, 260 + 32 * q : 260 + 32 * q + 32] = 1.0  # bc4
        wbias[sl, 0] = conv_b + LN_LA
        wbias[sl, 1] = LAM * conv_b
        wbias[sl, 2] = b1p + LN_LA
        wbias[sl, 3] = LAM * b1p
        wbias[sl, 4] = b2p + LN_LA
        wbias[sl, 5] = LAM * b2p
    wmat16 = wmat.astype(NP_BF16)

    # ---------------- per-core inputs + bag bookkeeping
    xf = x.reshape(n_total, R)
    seg_pad = np.concatenate([seg, np.full(n_pad, N_BAGS, seg.dtype)])
    in_maps = []
    bagA = np.zeros((N_CORES, T, QS), np.int64)
    bagB = np.full((N_CORES, T, QS), -1, np.int64)
    npad_sub = np.zeros((N_CORES, T, QS), np.int64)
    for c in range(N_CORES):
        s0 = c * NPAD
        xt = np.zeros((R, NPAD), np.float32)
        real = min(NPAD, max(0, n_total - s0))
        if real > 0:
            xt[:, :real] = xf[s0 : s0 + real].T
        sxr = np.zeros((SCHUNKS * 128, NPAD), NP_F8)
        sxr[:R] = xt.astype(NP_F8)
        sx = np.ascontiguousarray(
            sxr.reshape(SCHUNKS, 128, NPAD).transpose(1, 0, 2).reshape(
                128, SCHUNKS * NPAD
            )
        )
        ids = seg_pad[s0 : s0 + NPAD].reshape(T, QS, FD)
        first = ids[:, :, 0]
        # real-instance last bag per subtile (pad slots hold N_BAGS)
        real_mask = ids < N_BAGS
        last_real = np.where(real_mask, ids, -1).max(axis=2)
        if ((last_real - first) > 1).any() and (last_real >= 0).all():
            raise ValueError("subtile spans >2 bags; unsupported input shape")
        bagA[c] = np.where(first < N_BAGS, first, -1)
        hasB = (last_real > first) & (first < N_BAGS)
        bagB[c] = np.where(hasB, last_real, -1)
        npad_sub[c] = (~real_mask).sum(axis=2)
        maskA = (ids == first[:, :, None]).astype(np.float32)
        maskA *= real_mask  # pad slots excluded even if first is pad
        # layout [QS, T*FD]
        maskp = np.ascontiguousarray(
            maskA.transpose(1, 0, 2).reshape(QS, T * FD)
        ).astype(NP_BF16)
        in_maps.append(
            {
                "sx": sx,
                "wconv": wconv8,
                "wmat": wmat16,
                "wbias": wbias,
                "maskp": maskp,
            }
        )

    nc = _get_program(NPAD)
    trace_mode = int(os.environ.get("DEEPRC_TRACE", "0"))
    kwargs = {}
    if trace_mode == 1:
        kwargs = dict(trace=True, trace_cores=[0])
    elif trace_mode >= 2:
        kwargs = dict(trace=True, trace_cores=list(range(N_CORES)), stitch_traces=True)
    res = run_bass_kernel_spmd(
        nc,
        in_maps,
        core_ids=list(range(N_CORES)),
        **kwargs,
    )
    LAST_RESULTS = res

    # ---------------- pad-instance constants (host, float64-exact path)
    # a zero input row gives er4 = 0 (conv of zeros, max of zeros), so the
    # device computes for each pad slot: e_pad = selu(conv_b)+LA etc.  The
    # unmasked totals z/pooled include those; subtract exactly here.
    if n_pad > 0:
        e_pad = np.where(
            conv_b > 0, LAM * conv_b, LA * np.exp(conv_b) - LA
        ) + LA  # selu(conv_b) + LA, shape [K]
        hh = np.where(
            att_w1 @ (e_pad + C_SELU) + att_b1 > 0,
            LAM * (att_w1 @ (e_pad + C_SELU) + att_b1),
            LA * np.exp(att_w1 @ (e_pad + C_SELU) + att_b1) - LA,
        )
        hh2 = np.where(
            att_w2 @ hh + att_b2 > 0,
            LAM * (att_w2 @ hh + att_b2),
            LA * np.exp(att_w2 @ hh + att_b2) - LA,
        )
        att_pad = float(att_w3[0] @ hh2)
        pz_pad = float(np.exp(att_pad))
        ppool_pad = pz_pad * e_pad  # [K]
    else:
        pz_pad = 0.0
        ppool_pad = np.zeros(K)

    # ---------------- exact host combine (float64)
    Z = np.zeros(N_BAGS, np.float64)
    P = np.zeros((N_BAGS, K), np.float64)
    for c in range(N_CORES):
        r = res.results[c]
        outs = r["outs"].astype(np.float64)          # [128, 4T]
        pooled = outs[:, 0:T].reshape(QS, K, T)
        pooleda = outs[:, T : 2 * T].reshape(QS, K, T)
        z = outs[0:QS, 2 * T : 3 * T]                # [4, T]
        za = outs[0:QS, 3 * T : 4 * T]
        for j in range(T):
            for q in range(QS):
                bA = bagA[c, j, q]
                if bA < 0:
                    continue
                bB = bagB[c, j, q]
                if bB < 0:
                    Z[bA] += za[q, j]
                    P[bA] += pooleda[q, :, j]
                else:
                    Z[bA] += za[q, j]
                    P[bA] += pooleda[q, :, j]
                    npd = npad_sub[c, j, q]
                    Z[bB] += z[q, j] - za[q, j] - npd * pz_pad
                    P[bB] += (
                        pooled[q, :, j] - pooleda[q, :, j] - npd * ppool_pad
                    )

    out = np.zeros((N_BAGS, 1), np.float32)
    for b in range(N_BAGS):
        pooled_bag = P[b] / Z[b] + C_SELU
        out[b, 0] = np.float32(
            float(out_w.astype(np.float64)[0] @ pooled_bag) + float(out_b[0])
        )
    return out
